# revision 29
# baseline (speedup 1.0000x reference)
"""DenseSNN Trainium2 kernel: 4-layer LIF SNN, T=100 steps, B=128, D=H=2048, C=100.

Strategy
--------
Feed-forward unroll into per-layer phases (layer-l spikes at step t depend only
on layer-(l-1) spikes at steps <= t):

    CUR1 = x @ W1 + b1          (batched over all T*B rows)
    S1   = LIF-scan_T(CUR1)     (elementwise in (B,H), sequential in T)
    ... repeat for W2, W3; output = sum_t spikes of the Wo layer.

Pure data-parallel over batch: 16 samples/core on 8 cores, no collectives.

On-chip layout: activations transposed [feature -> 16 chunks x 128 partitions,
(t,b) -> free axis]. Matmuls are fp8e4 DoubleRow (2 k-tiles/instruction).
Weights are host-prescaled by 512; the PSUM->SBUF drain (scalar engine)
descales and adds the bias.

Performance structure (evolved over HW traces, 603us -> 383us):
- Spike tiles S are TIME-MAJOR [t][c][b], identical to the LIF scan's ring
  layout, so the spike export is a contiguous [128,1024] block per 4 steps,
  issued as a gpsimd-initiated DMA with bf16->fp8 cast (~620ns descriptor
  cost, data movement on otherwise-idle DMA engines). Matmuls read S through
  a 4D rhs access pattern [k, 2, t, b] (HW-verified bit-exact in DoubleRow).
- Row blocks per layer: 512 + 576 + 512 rows. The 576 block issues N=512 +
  N=64 matmul pairs sharing one LDWEIGHTS load (~275ns/group measured), so
  no weight-load stalls anywhere.
- The per-step scan ops are emitted as TWO independent feature-half chains
  (A: elems 0:128, B: 128:256) interleaved, so every DVE op's producer is
  >=2 instructions back; a distance-1 producer's completion-semaphore wait
  costs ~400-500ns exposed on this silicon.
- Block-wavefront emission: layer-(l+1) block-b matmuls are emitted right
  after layer-l block-b's scan so deep layers start as early as their data
  allows; (1,2) stays ahead of (2,0) so S1's buffer slot is free before S3
  (which shares it) is written.
- Output layer fast path: the no-reset membrane mhat (resets only subtract,
  so mhat >= true membrane) is computed with tensor_tensor_scan along time,
  16 lanes x 3 blocks. If max(mhat) stays under threshold there are provably
  zero output spikes and ssum (zero-initialized) is already the answer; the
  exact per-step scan runs in a Vector-engine conditional only if the
  threshold is approached (both paths HW-verified).
"""

import numpy as np
import ml_dtypes

import concourse.bass as bass
import concourse.bass_isa as bass_isa
import concourse.mybir as mybir
import concourse.tile as tile
from concourse import bacc
from concourse.bass_utils import run_bass_kernel_spmd

# Problem constants (hardcoded per contract)
T, B, D, H, C = 100, 128, 2048, 2048, 100
NCORES = 8
BC = B // NCORES          # 16 samples per core
R = T * BC                # 1600 rows (t,b) per core
KC = D // 128             # 16 contraction chunks
KP = KC // 2              # 8 DoubleRow chunk-pairs
HC = H // 128             # 16 output-feature chunks
BETA = 0.9
WSCALE = 512.0            # host-side fp8 weight scale; descaled at drain
RD = 8                    # spike ring depth (2 export blocks of 4 steps)

# Step blocks per layer: (step0, nsteps). Middle block carries the 64-row
# tail as an LDW-sharing matmul pair (N=512 + N=64).
BLOCKS = [(0, 32), (32, 36), (68, 32)]

F32 = mybir.dt.float32
BF16 = mybir.dt.bfloat16
F8 = mybir.dt.float8e4
ALU = mybir.AluOpType
ACTF = mybir.ActivationFunctionType
DROW = mybir.MatmulPerfMode.DoubleRow


def _build_nc():
    nc = bacc.Bacc("TRN2", target_bir_lowering=False)

    xT_d = nc.dram_tensor("xT", [KC, 128, R], F8, kind="ExternalInput")
    w_d = [
        nc.dram_tensor("w1", [D, H], F8, kind="ExternalInput"),
        nc.dram_tensor("w2", [H, H], F8, kind="ExternalInput"),
        nc.dram_tensor("w3", [H, H], F8, kind="ExternalInput"),
    ]
    wo_d = nc.dram_tensor("wo", [H, 128], F8, kind="ExternalInput")  # C pad 128
    bias_d = nc.dram_tensor("biases", [128, 3 * HC], F32, kind="ExternalInput")
    bo_d = nc.dram_tensor("biaso", [C, 1], F32, kind="ExternalInput")
    out_d = nc.dram_tensor("out", [C, BC], F32, kind="ExternalOutput")

    with tile.TileContext(nc) as tc:
        with (
            tc.tile_pool(name="spool", bufs=2) as spool,
            tc.tile_pool(name="wpool", bufs=2) as wpool,
            tc.tile_pool(name="xpool", bufs=1) as xpool,
            tc.tile_pool(name="cpool", bufs=2) as cpool,
            tc.tile_pool(name="opool", bufs=3) as opool,
            tc.tile_pool(name="small", bufs=1) as small,
            tc.tile_pool(name="psb", bufs=6, space="PSUM") as psb,
            tc.tile_pool(name="pst", bufs=2, space="PSUM") as pst,
        ):
            # Spike tensors, TIME-MAJOR: [128part(k-in), (t, c, b)] fp8.
            # S3 reuses S1's slot (S1 is dead once layer-2 matmuls finish).
            S1 = spool.tile([128, T * KC * BC], F8, tag="S")  # 25600/partition
            S2 = spool.tile([128, T * KC * BC], F8, tag="S")
            S3 = spool.tile([128, T * KC * BC], F8, tag="S")
            w_sb = [
                wpool.tile([128, KC * H], F8, tag="W", name=f"w{i}_sb")
                for i in range(3)
            ]                                  # w3 reuses w1's slot
            wo_sb = small.tile([128, KC * 128], F8)

            # LIF state (bf16, 2x DVE mode). Ring is 8 deep; export copies a
            # contiguous 4-slot block to S every 4 steps.
            stm = small.tile([128, 9 * 256], BF16)   # 3 layers x (2 pp + tmp)
            mem_pp = [
                [stm[:, (3 * li + pp) * 256:(3 * li + pp + 1) * 256]
                 for pp in range(2)]
                for li in range(3)
            ]
            mem_t = [
                stm[:, (3 * li + 2) * 256:(3 * li + 3) * 256] for li in range(3)
            ]
            sring = small.tile([128, 3 * RD * 256], BF16)

            # fp32 small state: biases + output layer
            st = small.tile([128, 224], F32)
            bias_sb = st[:, 0:48]               # [128,48] = 3 layers x 16 chunks
            memo = st[:100, 48:64]              # [100, 16]
            ssum = st[:100, 64:80]
            bo_sb = st[:100, 80:81]             # [100, 1]
            mxb = st[:100, 88:91]               # per-block max of mhat
            mx1 = st[:100, 91:92]
            mxa = st[:100, 92:93]
            so_ring = st[:100, 96:224]          # [100, 8*16] spike ring
            # no-reset output membrane (b-major: [b, t]) + beta constant row
            mhat = small.tile([128, BC * T], F32)
            betat = small.tile([128, 36], F32)

            nc.gpsimd.memset(st[:], 0.0)
            nc.gpsimd.memset(stm[:], 0.0)
            nc.gpsimd.memset(sring[:], 0.0)
            nc.gpsimd.memset(mhat[:], 0.0)
            nc.gpsimd.memset(betat[:], BETA)
            nc.sync.dma_start(bias_sb, bias_d[:])
            nc.sync.dma_start(bo_sb, bo_d[:])
            for kc in range(KC):
                nc.sync.dma_start(
                    wo_sb[:, kc * 128:(kc + 1) * 128],
                    wo_d[kc * 128:(kc + 1) * 128, :],
                )

            def load_w(li):
                w = w_sb[li]
                for kc in range(KC):
                    nc.sync.dma_start(
                        w[:, kc * H:(kc + 1) * H],
                        w_d[li][kc * 128:(kc + 1) * 128, :],
                    )

            def mm_group(w3d, rhs_big, rhs_tail, drain_big, drain_tail, nhc,
                         nst):
                """Matmuls + drains for one (layer, block). nhc output chunks;
                block has nst steps: 32 big-only, or 36 = 32 big + 4 tail."""
                nr = 512
                for hc in range(nhc):
                    lo, hi = hc * 128, hc * 128 + 128
                    ps = psb.tile([128, nr], F32, tag="ps", name="ps")
                    pt = None
                    if nst == 36:
                        pt = pst.tile([128, 64], F32, tag="pt", name="pt")
                    for kp in range(KP):
                        nc.tensor.matmul(
                            ps[:], w3d[:, 2 * kp:2 * kp + 2, lo:hi],
                            rhs_big(kp),
                            start=(kp == 0), stop=(kp == KP - 1),
                            perf_mode=DROW,
                        )
                        if pt is not None:
                            nc.tensor.matmul(
                                pt[:], w3d[:, 2 * kp:2 * kp + 2, lo:hi],
                                rhs_tail(kp),
                                start=(kp == 0), stop=(kp == KP - 1),
                                perf_mode=DROW,
                            )
                    drain_big(hc, ps)
                    if pt is not None:
                        drain_tail(hc, pt)

            def hidden_block(li, bi, rhs_of, S_out):
                """One (hidden layer, block): matmuls + LIF scan steps."""
                mpp = mem_pp[li]
                mt = mem_t[li]
                ring = sring[:, li * RD * 256:(li + 1) * RD * 256]
                w3d = w_sb[li].rearrange("p (c h) -> p c h", c=KC)
                t0, nst = BLOCKS[bi]
                rhs_big, rhs_tail = rhs_of(bi)
                cur = cpool.tile([128, 36 * 256], BF16, tag="cur", name="cur")
                curT = cur.rearrange("p (t cb) -> p t cb", cb=256)

                def drain_big(hc, ps):
                    nc.scalar.activation(
                        curT[:, :32, hc * BC:(hc + 1) * BC],
                        ps[:].rearrange("p (t b) -> p t b", b=BC),
                        ACTF.Identity,
                        bias=bias_sb[:, li * HC + hc:li * HC + hc + 1],
                        scale=1.0 / WSCALE,
                    )

                def drain_tail(hc, pt):
                    nc.scalar.activation(
                        curT[:, 32:36, hc * BC:(hc + 1) * BC],
                        pt[:].rearrange("p (t b) -> p t b", b=BC),
                        ACTF.Identity,
                        bias=bias_sb[:, li * HC + hc:li * HC + hc + 1],
                        scale=1.0 / WSCALE,
                    )

                mm_group(w3d, rhs_big, rhs_tail, drain_big, drain_tail,
                         HC, nst)

                for tl in range(nst):
                    t = t0 + tl
                    cur_t = cur[:, tl * 256:(tl + 1) * 256]
                    sp_c = ring[:, ((t - 1) % RD) * 256:
                                ((t - 1) % RD + 1) * 256]
                    sn_c = ring[:, (t % RD) * 256:(t % RD + 1) * 256]
                    m_prev = mpp[(t - 1) % 2]
                    m_cur = mpp[t % 2]
                    # Two independent feature-half chains (A: elems 0:128,
                    # B: 128:256) interleaved so every DVE op's producer is
                    # >=2 instructions back — its completion-semaphore wait
                    # is then already satisfied when the op reaches the
                    # engine head (a distance-1 wait costs ~400ns exposed).
                    hv = [(0, 128), (128, 256)]
                    for lo, hi in hv:
                        # tmp = beta*mem + cur
                        nc.vector.scalar_tensor_tensor(
                            mt[:, lo:hi], m_prev[:, lo:hi], BETA,
                            cur_t[:, lo:hi], ALU.mult, ALU.add
                        )
                    for eng, (lo, hi) in zip((nc.vector, nc.gpsimd), hv):
                        # mem_new = tmp - s_prev (reset-by-subtraction;
                        # ring slot 7 holds zeros at t=0). The B-half runs
                        # on gpsimd (TensorTensor is Pool-legal) to shed
                        # ~17% of the DVE's per-step work.
                        eng.tensor_tensor(
                            m_cur[:, lo:hi], mt[:, lo:hi], sp_c[:, lo:hi],
                            ALU.subtract
                        )
                    for lo, hi in hv:
                        # spike = mem_new > 1
                        nc.vector.tensor_scalar(
                            sn_c[:, lo:hi], m_cur[:, lo:hi], 1.0, None,
                            ALU.is_gt
                        )
                    if t % 4 == 3:
                        # contiguous 4-step block export to time-major S:
                        # gpsimd-initiated DMA with bf16->fp8 cast (data
                        # movement rides the idle DMA engines)
                        blk = ((t // 4) % 2) * 1024
                        nc.gpsimd.dma_start(
                            S_out[:, (t - 3) * 256:(t + 1) * 256],
                            ring[:, blk:blk + 1024],
                        )

            # ---- rhs providers
            def rhs_of_l1(bi):
                t0, nst = BLOCKS[bi]
                r0 = t0 * BC
                nr_all = nst * BC
                xin = xpool.tile([128, KC * 576], F8, tag="xin", name="xin")
                for kc in range(KC):
                    nc.sync.dma_start(
                        xin[:, kc * nr_all:(kc + 1) * nr_all],
                        xT_d[kc][:, r0:r0 + nr_all],
                    )
                x3 = xin[:, :KC * nr_all].rearrange("p (c r) -> p c r", c=KC)
                rhs_big = lambda kp: x3[:, 2 * kp:2 * kp + 2, 0:512]
                rhs_tail = lambda kp: x3[:, 2 * kp:2 * kp + 2, 512:576]
                return rhs_big, rhs_tail

            def rhs_of_S(S_in):
                S4 = S_in.rearrange("p (t c b) -> p c t b", t=T, c=KC)

                def f(bi):
                    t0, nst = BLOCKS[bi]
                    rhs_big = lambda kp: S4[:, 2 * kp:2 * kp + 2,
                                            t0:t0 + 32, :]
                    rhs_tail = lambda kp: S4[:, 2 * kp:2 * kp + 2,
                                             t0 + 32:t0 + 36, :]
                    return rhs_big, rhs_tail
                return f

            # ---- network
            # DMA order: xin block 0 first (small, gates the first matmul),
            # then w1; w2/w3 stream during earlier layers' compute.
            rhs_l1_cache = {}

            def rhs_of_l1_cached(bi):
                if bi not in rhs_l1_cache:
                    rhs_l1_cache[bi] = rhs_of_l1(bi)
                return rhs_l1_cache[bi]

            curo_tiles = {}

            def out_block(bi):
                """Output layer for one block: matmul pair + the no-reset
                membrane prefix-scan (fast path). The true LIF scan only
                runs in the conditional fallback if the no-reset membrane
                ever crosses threshold (resets can only lower it, so no
                crossing there proves zero spikes exactly)."""
                t0, nst = BLOCKS[bi]
                wo3d = wo_sb.rearrange("p (c h) -> p c h", c=KC)
                S3_4 = S3.rearrange("p (t c b) -> p c t b", t=T, c=KC)
                rhs_big = lambda kp: S3_4[:, 2 * kp:2 * kp + 2, t0:t0 + 32, :]
                rhs_tail = lambda kp: S3_4[:, 2 * kp:2 * kp + 2,
                                           t0 + 32:t0 + 36, :]
                curo = opool.tile([128, 576], BF16, tag="curo", name="curo")
                curo_tiles[bi] = curo

                def drain_big(hc, ps):
                    nc.scalar.activation(
                        curo[:100, 0:512], ps[:100, :], ACTF.Identity,
                        bias=bo_sb, scale=1.0 / WSCALE,
                    )

                def drain_tail(hc, pt):
                    nc.scalar.activation(
                        curo[:100, 512:576], pt[:100, :], ACTF.Identity,
                        bias=bo_sb, scale=1.0 / WSCALE,
                    )

                mm_group(wo3d, rhs_big, rhs_tail, drain_big, drain_tail,
                         1, nst)

                # mhat[b, t] = beta*mhat[b, t-1] + curo[t, b] (per-lane scan
                # along time, chained across blocks via `initial`)
                curo3 = curo.rearrange("p (t b) -> p t b", b=BC)
                for b in range(BC):
                    nc.vector.tensor_tensor_scan(
                        mhat[:100, b * T + t0:b * T + t0 + nst],
                        betat[:100, :nst],
                        curo3[:100, :nst, b],
                        0.0 if bi == 0 else
                        mhat[:100, b * T + t0 - 1:b * T + t0],
                        ALU.mult, ALU.add,
                    )
                mhat3 = mhat.rearrange("p (b t) -> p b t", b=BC)
                nc.vector.tensor_reduce(
                    mxb[:, bi:bi + 1], mhat3[:100, :, t0:t0 + nst],
                    mybir.AxisListType.XY, ALU.max,
                )

            def lo_scan_fallback():
                for bi, (t0, nst) in enumerate(BLOCKS):
                    curo = curo_tiles[bi]
                    for tl in range(nst):
                        t = t0 + tl
                        cur_t = curo[:100, tl * BC:(tl + 1) * BC]
                        so_prev = so_ring[:, ((t - 1) % 8) * BC:
                                          ((t - 1) % 8 + 1) * BC]
                        so_new = so_ring[:, (t % 8) * BC:(t % 8 + 1) * BC]
                        nc.vector.scalar_tensor_tensor(
                            memo, memo, BETA, cur_t, ALU.mult, ALU.add
                        )
                        nc.vector.scalar_tensor_tensor(
                            so_new, memo, 1.0, so_prev, ALU.subtract,
                            ALU.is_gt
                        )
                        nc.vector.tensor_tensor(
                            memo, memo, so_prev, ALU.subtract
                        )
                        nc.vector.tensor_tensor(
                            ssum, ssum, so_new, ALU.add
                        )

            # Wavefront emission: layer-(l+1) block-b matmuls right after
            # layer-l block-b's scan, so deep layers start as early as their
            # dependencies allow. (1,2) stays ahead of (2,0) so S1 is dead
            # before S3 (sharing its slot) is written.
            rhs_of_l1_cached(0)
            load_w(0)
            load_w(1)
            rhs_S1 = rhs_of_S(S1)
            rhs_S2 = rhs_of_S(S2)

            hidden_block(0, 0, rhs_of_l1_cached, S1)    # L1 B0
            hidden_block(0, 1, rhs_of_l1_cached, S1)    # L1 B1
            hidden_block(1, 0, rhs_S1, S2)              # L2 B0
            hidden_block(0, 2, rhs_of_l1_cached, S1)    # L1 B2
            load_w(2)                                   # w3 into w1's slot
            hidden_block(1, 1, rhs_S1, S2)              # L2 B1
            hidden_block(1, 2, rhs_S1, S2)              # L2 B2 (frees S1 slot)
            hidden_block(2, 0, rhs_S2, S3)              # L3 B0
            hidden_block(2, 1, rhs_S2, S3)              # L3 B1
            out_block(0)                                # Lo B0
            hidden_block(2, 2, rhs_S2, S3)              # L3 B2
            out_block(1)                                # Lo B1
            out_block(2)                                # Lo B2

            # max over blocks, then over partitions; spikes are possible
            # only if the no-reset membrane crossed ~threshold somewhere
            nc.vector.tensor_reduce(
                mx1, mxb, mybir.AxisListType.X, ALU.max
            )
            nc.gpsimd.partition_all_reduce(
                mxa, mx1, 100, bass_isa.ReduceOp.max
            )
            # Engine-level conditional on the Vector engine (the whole
            # fallback is DVE-only): int32 bit-pattern compare -- positive
            # floats order like ints, negative floats read as negative ints
            # (condition false either way). 0.9375f leaves margin under the
            # spike threshold 1.0. (Fallback path HW-verified by forcing
            # the threshold to 0: identical output.)
            with tc.tile_critical():
                vreg = nc.vector.alloc_register("lomax_bits")
                nc.vector.reg_load(
                    vreg, mxa[0:1, 0:1].bitcast(mybir.dt.int32)
                )
                with nc.vector.If(nc.vector.snap(vreg) > 0x3F700000):
                    lo_scan_fallback()

            nc.sync.dma_start(out_d[:], ssum)

    nc.compile()
    return nc


_NC_CACHE = None


def _get_nc():
    global _NC_CACHE
    if _NC_CACHE is None:
        _NC_CACHE = _build_nc()
    return _NC_CACHE


def make_in_maps(x_seq, W1, b1, W2, b2, W3, b3, Wo, bo):
    f8 = ml_dtypes.float8_e4m3
    w1 = np.ascontiguousarray((W1 * WSCALE).astype(f8))
    w2 = np.ascontiguousarray((W2 * WSCALE).astype(f8))
    w3 = np.ascontiguousarray((W3 * WSCALE).astype(f8))
    wo_pad = np.zeros((H, 128), np.float32)
    wo_pad[:, :C] = Wo * WSCALE
    wo = np.ascontiguousarray(wo_pad.astype(f8))
    biases = np.concatenate(
        [b.reshape(HC, 128).T for b in (b1, b2, b3)], axis=1
    ).astype(np.float32)                       # [128, 48]
    biases = np.ascontiguousarray(biases)
    bo_a = np.ascontiguousarray(bo.reshape(C, 1).astype(np.float32))
    in_maps = []
    for c in range(NCORES):
        xs = x_seq[:, c * BC:(c + 1) * BC, :]              # [T, BC, D]
        xT = xs.transpose(2, 0, 1).reshape(KC, 128, R)     # [D,(t,b)] chunked
        in_maps.append({
            "xT": np.ascontiguousarray(xT.astype(f8)),
            "w1": w1, "w2": w2, "w3": w3, "wo": wo,
            "biases": biases, "biaso": bo_a,
        })
    return in_maps


def kernel(x_seq, W1, b1, W2, b2, W3, b3, Wo, bo):
    nc = _get_nc()
    in_maps = make_in_maps(x_seq, W1, b1, W2, b2, W3, b3, Wo, bo)
    res = run_bass_kernel_spmd(nc, in_maps, core_ids=list(range(NCORES)))
    outs = [res.results[c]["out"] for c in range(NCORES)]   # each [C, BC]
    return np.concatenate([o.T for o in outs], axis=0).astype(np.float32)


# revision 31
# speedup vs baseline: 1.2646x; 1.2646x over previous
"""DenseSNN Trainium2 kernel: 4-layer LIF SNN, T=100 steps, B=128, D=H=2048, C=100.

Strategy
--------
Feed-forward unroll into per-layer phases (layer-l spikes at step t depend only
on layer-(l-1) spikes at steps <= t):

    CUR1 = x @ W1 + b1          (batched over all T*B rows)
    S1   = LIF-scan_T(CUR1)     (elementwise in (B,H), sequential in T)
    ... repeat for W2, W3; output = sum_t spikes of the Wo layer.

Pure data-parallel over batch: 16 samples/core on 8 cores, no collectives.

On-chip layout: activations transposed [feature -> 16 chunks x 128 partitions,
(t,b) -> free axis]. Matmuls are fp8e4 DoubleRow (2 k-tiles/instruction).
Weights are host-prescaled by 512; the PSUM->SBUF drain (scalar engine)
descales and adds the bias.

Performance structure (evolved over HW traces, 603us -> 383us):
- Spike tiles S are TIME-MAJOR [t][c][b], identical to the LIF scan's ring
  layout, so the spike export is a contiguous [128,1024] block per 4 steps,
  issued as a gpsimd-initiated DMA with bf16->fp8 cast (~620ns descriptor
  cost, data movement on otherwise-idle DMA engines). Matmuls read S through
  a 4D rhs access pattern [k, 2, t, b] (HW-verified bit-exact in DoubleRow).
- Row blocks per layer: 512 + 576 + 512 rows. The 576 block issues N=512 +
  N=64 matmul pairs sharing one LDWEIGHTS load (~275ns/group measured), so
  no weight-load stalls anywhere.
- The per-step scan ops are emitted as TWO independent feature-half chains
  (A: elems 0:128, B: 128:256) interleaved, so every DVE op's producer is
  >=2 instructions back; a distance-1 producer's completion-semaphore wait
  costs ~400-500ns exposed on this silicon.
- Block-wavefront emission: layer-(l+1) block-b matmuls are emitted right
  after layer-l block-b's scan so deep layers start as early as their data
  allows; (1,2) stays ahead of (2,0) so S1's buffer slot is free before S3
  (which shares it) is written.
- Output layer fast path: the no-reset membrane mhat (resets only subtract,
  so mhat >= true membrane) is computed with tensor_tensor_scan along time,
  16 lanes x 3 blocks. If max(mhat) stays under threshold there are provably
  zero output spikes and ssum (zero-initialized) is already the answer; the
  exact per-step scan runs in a Vector-engine conditional only if the
  threshold is approached (both paths HW-verified).
"""

import numpy as np
import ml_dtypes

import concourse.bass as bass
import concourse.bass_isa as bass_isa
import concourse.mybir as mybir
import concourse.tile as tile
from concourse import bacc
from concourse.bass_utils import run_bass_kernel_spmd

# Problem constants (hardcoded per contract)
T, B, D, H, C = 100, 128, 2048, 2048, 100
NCORES = 8
BC = B // NCORES          # 16 samples per core
R = T * BC                # 1600 rows (t,b) per core
KC = D // 128             # 16 contraction chunks
KP = KC // 2              # 8 DoubleRow chunk-pairs
HC = H // 128             # 16 output-feature chunks
BETA = 0.9
WSCALE = 512.0            # host-side fp8 weight scale; descaled at drain
RD = 8                    # spike ring depth (2 export blocks of 4 steps)

# Step blocks per layer: (step0, nsteps). Middle block carries the 64-row
# tail as an LDW-sharing matmul pair (N=512 + N=64).
BLOCKS = [(0, 32), (32, 36), (68, 32)]

F32 = mybir.dt.float32
BF16 = mybir.dt.bfloat16
F8 = mybir.dt.float8e4
ALU = mybir.AluOpType
ACTF = mybir.ActivationFunctionType
DROW = mybir.MatmulPerfMode.DoubleRow


def _build_nc():
    nc = bacc.Bacc("TRN2", target_bir_lowering=False)

    xT_d = nc.dram_tensor("xT", [KC, 128, R], F8, kind="ExternalInput")
    w_d = [
        nc.dram_tensor("w1", [D, H], F8, kind="ExternalInput"),
        nc.dram_tensor("w2", [H, H], F8, kind="ExternalInput"),
        nc.dram_tensor("w3", [H, H], F8, kind="ExternalInput"),
    ]
    wo_d = nc.dram_tensor("wo", [H, 128], F8, kind="ExternalInput")  # C pad 128
    bias_d = nc.dram_tensor("biases", [128, 3 * HC], F32, kind="ExternalInput")
    bo_d = nc.dram_tensor("biaso", [C, 1], F32, kind="ExternalInput")
    out_d = nc.dram_tensor("out", [C, BC], F32, kind="ExternalOutput")

    with tile.TileContext(nc) as tc:
        with (
            tc.tile_pool(name="spool", bufs=2) as spool,
            tc.tile_pool(name="wpool", bufs=2) as wpool,
            tc.tile_pool(name="xpool", bufs=1) as xpool,
            tc.tile_pool(name="cpool", bufs=2) as cpool,
            tc.tile_pool(name="opool", bufs=3) as opool,
            tc.tile_pool(name="small", bufs=1) as small,
            tc.tile_pool(name="psb", bufs=6, space="PSUM") as psb,
            tc.tile_pool(name="pst", bufs=2, space="PSUM") as pst,
        ):
            # Spike tensors, TIME-MAJOR: [128part(k-in), (t, c, b)] fp8.
            # S3 reuses S1's slot (S1 is dead once layer-2 matmuls finish).
            S1 = spool.tile([128, T * KC * BC], F8, tag="S")  # 25600/partition
            S2 = spool.tile([128, T * KC * BC], F8, tag="S")
            S3 = spool.tile([128, T * KC * BC], F8, tag="S")
            w_sb = [
                wpool.tile([128, KC * H], F8, tag="W", name=f"w{i}_sb")
                for i in range(3)
            ]                                  # w3 reuses w1's slot
            wo_sb = small.tile([128, KC * 128], F8)

            # LIF state (bf16, 2x DVE mode). Ring is 8 deep; export copies a
            # contiguous 4-slot block to S every 4 steps.
            stm = small.tile([128, 9 * 256], BF16)   # 3 layers x (2 pp + tmp)
            mem_pp = [
                [stm[:, (3 * li + pp) * 256:(3 * li + pp + 1) * 256]
                 for pp in range(2)]
                for li in range(3)
            ]
            mem_t = [
                stm[:, (3 * li + 2) * 256:(3 * li + 3) * 256] for li in range(3)
            ]
            sring = small.tile([128, 3 * RD * 256], BF16)

            # fp32 small state: biases + output layer
            st = small.tile([128, 224], F32)
            bias_sb = st[:, 0:48]               # [128,48] = 3 layers x 16 chunks
            memo = st[:100, 48:64]              # [100, 16]
            ssum = st[:100, 64:80]
            bo_sb = st[:100, 80:81]             # [100, 1]
            mxb = st[:100, 88:91]               # per-block max of mhat
            mx1 = st[:100, 91:92]
            mxa = st[:100, 92:93]
            so_ring = st[:100, 96:224]          # [100, 8*16] spike ring
            # no-reset output membrane (b-major: [b, t]) + beta constant row
            mhat = small.tile([128, BC * T], F32)
            betat = small.tile([128, 36], F32)

            nc.gpsimd.memset(st[:], 0.0)
            nc.gpsimd.memset(stm[:], 0.0)
            nc.gpsimd.memset(sring[:], 0.0)
            nc.gpsimd.memset(mhat[:], 0.0)
            nc.gpsimd.memset(betat[:], BETA)
            nc.sync.dma_start(bias_sb, bias_d[:])
            nc.sync.dma_start(bo_sb, bo_d[:])
            for kc in range(KC):
                nc.sync.dma_start(
                    wo_sb[:, kc * 128:(kc + 1) * 128],
                    wo_d[kc * 128:(kc + 1) * 128, :],
                )

            def load_w(li):
                w = w_sb[li]
                for kc in range(KC):
                    nc.sync.dma_start(
                        w[:, kc * H:(kc + 1) * H],
                        w_d[li][kc * 128:(kc + 1) * 128, :],
                    )

            def mm_group(w3d, rhs_big, rhs_tail, drain_big, drain_tail, nhc,
                         nst):
                """Matmuls + drains for one (layer, block). nhc output chunks;
                block has nst steps: 32 big-only, or 36 = 32 big + 4 tail."""
                nr = 512
                for hc in range(nhc):
                    lo, hi = hc * 128, hc * 128 + 128
                    ps = psb.tile([128, nr], F32, tag="ps", name="ps")
                    pt = None
                    if nst == 36:
                        pt = pst.tile([128, 64], F32, tag="pt", name="pt")
                    for kp in range(KP):
                        nc.tensor.matmul(
                            ps[:], w3d[:, 2 * kp:2 * kp + 2, lo:hi],
                            rhs_big(kp),
                            start=(kp == 0), stop=(kp == KP - 1),
                            perf_mode=DROW,
                        )
                        if pt is not None:
                            nc.tensor.matmul(
                                pt[:], w3d[:, 2 * kp:2 * kp + 2, lo:hi],
                                rhs_tail(kp),
                                start=(kp == 0), stop=(kp == KP - 1),
                                perf_mode=DROW,
                            )
                    drain_big(hc, ps)
                    if pt is not None:
                        drain_tail(hc, pt)

            def hidden_block(li, bi, rhs_of, S_out, filler=None):
                """One (hidden layer, block): matmuls + LIF scan steps.
                filler: optional list of closures (independent DVE work)
                emitted one per scan step to absorb solo-chain sem-wait
                latency."""
                mpp = mem_pp[li]
                mt = mem_t[li]
                ring = sring[:, li * RD * 256:(li + 1) * RD * 256]
                w3d = w_sb[li].rearrange("p (c h) -> p c h", c=KC)
                t0, nst = BLOCKS[bi]
                rhs_big, rhs_tail = rhs_of(bi)
                cur = cpool.tile([128, 36 * 256], BF16, tag="cur", name="cur")
                curT = cur.rearrange("p (t cb) -> p t cb", cb=256)

                def drain_big(hc, ps):
                    nc.scalar.activation(
                        curT[:, :32, hc * BC:(hc + 1) * BC],
                        ps[:].rearrange("p (t b) -> p t b", b=BC),
                        ACTF.Identity,
                        bias=bias_sb[:, li * HC + hc:li * HC + hc + 1],
                        scale=1.0 / WSCALE,
                    )

                def drain_tail(hc, pt):
                    nc.scalar.activation(
                        curT[:, 32:36, hc * BC:(hc + 1) * BC],
                        pt[:].rearrange("p (t b) -> p t b", b=BC),
                        ACTF.Identity,
                        bias=bias_sb[:, li * HC + hc:li * HC + hc + 1],
                        scale=1.0 / WSCALE,
                    )

                mm_group(w3d, rhs_big, rhs_tail, drain_big, drain_tail,
                         HC, nst)

                for tl in range(nst):
                    t = t0 + tl
                    cur_t = cur[:, tl * 256:(tl + 1) * 256]
                    sp_c = ring[:, ((t - 1) % RD) * 256:
                                ((t - 1) % RD + 1) * 256]
                    sn_c = ring[:, (t % RD) * 256:(t % RD + 1) * 256]
                    m_prev = mpp[(t - 1) % 2]
                    m_cur = mpp[t % 2]
                    # Two independent feature-half chains (A: elems 0:128,
                    # B: 128:256) interleaved so every DVE op's producer is
                    # >=2 instructions back — its completion-semaphore wait
                    # is then already satisfied when the op reaches the
                    # engine head (a distance-1 wait costs ~400ns exposed).
                    hv = [(0, 128), (128, 256)]
                    for lo, hi in hv:
                        # tmp = beta*mem + cur
                        nc.vector.scalar_tensor_tensor(
                            mt[:, lo:hi], m_prev[:, lo:hi], BETA,
                            cur_t[:, lo:hi], ALU.mult, ALU.add
                        )
                    for lo, hi in hv:
                        # mem_new = tmp - s_prev (reset-by-subtraction;
                        # ring slot 7 holds zeros at t=0)
                        nc.vector.tensor_tensor(
                            m_cur[:, lo:hi], mt[:, lo:hi], sp_c[:, lo:hi],
                            ALU.subtract
                        )
                    for lo, hi in hv:
                        # spike = mem_new > 1
                        nc.vector.tensor_scalar(
                            sn_c[:, lo:hi], m_cur[:, lo:hi], 1.0, None,
                            ALU.is_gt
                        )
                    if filler:
                        filler.pop(0)()
                    if t % 4 == 3:
                        # contiguous 4-step block export to time-major S:
                        # gpsimd-initiated DMA with bf16->fp8 cast (data
                        # movement rides the idle DMA engines)
                        blk = ((t // 4) % 2) * 1024
                        nc.gpsimd.dma_start(
                            S_out[:, (t - 3) * 256:(t + 1) * 256],
                            ring[:, blk:blk + 1024],
                        )

            # ---- rhs providers
            def rhs_of_l1(bi):
                t0, nst = BLOCKS[bi]
                r0 = t0 * BC
                nr_all = nst * BC
                xin = xpool.tile([128, KC * 576], F8, tag="xin", name="xin")
                for kc in range(KC):
                    nc.sync.dma_start(
                        xin[:, kc * nr_all:(kc + 1) * nr_all],
                        xT_d[kc][:, r0:r0 + nr_all],
                    )
                x3 = xin[:, :KC * nr_all].rearrange("p (c r) -> p c r", c=KC)
                rhs_big = lambda kp: x3[:, 2 * kp:2 * kp + 2, 0:512]
                rhs_tail = lambda kp: x3[:, 2 * kp:2 * kp + 2, 512:576]
                return rhs_big, rhs_tail

            def rhs_of_S(S_in):
                S4 = S_in.rearrange("p (t c b) -> p c t b", t=T, c=KC)

                def f(bi):
                    t0, nst = BLOCKS[bi]
                    rhs_big = lambda kp: S4[:, 2 * kp:2 * kp + 2,
                                            t0:t0 + 32, :]
                    rhs_tail = lambda kp: S4[:, 2 * kp:2 * kp + 2,
                                             t0 + 32:t0 + 36, :]
                    return rhs_big, rhs_tail
                return f

            # ---- network
            # DMA order: xin block 0 first (small, gates the first matmul),
            # then w1; w2/w3 stream during earlier layers' compute.
            rhs_l1_cache = {}

            def rhs_of_l1_cached(bi):
                if bi not in rhs_l1_cache:
                    rhs_l1_cache[bi] = rhs_of_l1(bi)
                return rhs_l1_cache[bi]

            curo_tiles = {}

            def out_block(bi, defer=False):
                """Output layer for one block: matmul pair + the no-reset
                membrane prefix-scan (fast path). The true LIF scan only
                runs in the conditional fallback if the no-reset membrane
                ever crosses threshold (resets can only lower it, so no
                crossing there proves zero spikes exactly)."""
                t0, nst = BLOCKS[bi]
                wo3d = wo_sb.rearrange("p (c h) -> p c h", c=KC)
                S3_4 = S3.rearrange("p (t c b) -> p c t b", t=T, c=KC)
                rhs_big = lambda kp: S3_4[:, 2 * kp:2 * kp + 2, t0:t0 + 32, :]
                rhs_tail = lambda kp: S3_4[:, 2 * kp:2 * kp + 2,
                                           t0 + 32:t0 + 36, :]
                curo = opool.tile([128, 576], BF16, tag="curo", name="curo")
                curo_tiles[bi] = curo

                def drain_big(hc, ps):
                    nc.scalar.activation(
                        curo[:100, 0:512], ps[:100, :], ACTF.Identity,
                        bias=bo_sb, scale=1.0 / WSCALE,
                    )

                def drain_tail(hc, pt):
                    nc.scalar.activation(
                        curo[:100, 512:576], pt[:100, :], ACTF.Identity,
                        bias=bo_sb, scale=1.0 / WSCALE,
                    )

                mm_group(wo3d, rhs_big, rhs_tail, drain_big, drain_tail,
                         1, nst)

                # mhat[b, t] = beta*mhat[b, t-1] + curo[t, b] (per-lane scan
                # along time, chained across blocks via `initial`)
                curo3 = curo.rearrange("p (t b) -> p t b", b=BC)

                def tts_lane(b):
                    nc.vector.tensor_tensor_scan(
                        mhat[:100, b * T + t0:b * T + t0 + nst],
                        betat[:100, :nst],
                        curo3[:100, :nst, b],
                        0.0 if bi == 0 else
                        mhat[:100, b * T + t0 - 1:b * T + t0],
                        ALU.mult, ALU.add,
                    )

                def blk_reduce():
                    mhat3 = mhat.rearrange("p (b t) -> p b t", b=BC)
                    nc.vector.tensor_reduce(
                        mxb[:, bi:bi + 1], mhat3[:100, :, t0:t0 + nst],
                        mybir.AxisListType.XY, ALU.max,
                    )

                closures = [lambda b=b: tts_lane(b) for b in range(BC)]
                closures.append(blk_reduce)
                if defer:
                    return closures
                for c in closures:
                    c()

            def lo_scan_fallback():
                for bi, (t0, nst) in enumerate(BLOCKS):
                    curo = curo_tiles[bi]
                    for tl in range(nst):
                        t = t0 + tl
                        cur_t = curo[:100, tl * BC:(tl + 1) * BC]
                        so_prev = so_ring[:, ((t - 1) % 8) * BC:
                                          ((t - 1) % 8 + 1) * BC]
                        so_new = so_ring[:, (t % 8) * BC:(t % 8 + 1) * BC]
                        nc.vector.scalar_tensor_tensor(
                            memo, memo, BETA, cur_t, ALU.mult, ALU.add
                        )
                        nc.vector.scalar_tensor_tensor(
                            so_new, memo, 1.0, so_prev, ALU.subtract,
                            ALU.is_gt
                        )
                        nc.vector.tensor_tensor(
                            memo, memo, so_prev, ALU.subtract
                        )
                        nc.vector.tensor_tensor(
                            ssum, ssum, so_new, ALU.add
                        )

            # Wavefront emission: layer-(l+1) block-b matmuls right after
            # layer-l block-b's scan, so deep layers start as early as their
            # dependencies allow. (1,2) stays ahead of (2,0) so S1 is dead
            # before S3 (sharing its slot) is written.
            rhs_of_l1_cached(0)
            load_w(0)
            load_w(1)
            rhs_S1 = rhs_of_S(S1)
            rhs_S2 = rhs_of_S(S2)

            hidden_block(0, 0, rhs_of_l1_cached, S1)    # L1 B0
            hidden_block(0, 1, rhs_of_l1_cached, S1)    # L1 B1
            hidden_block(1, 0, rhs_S1, S2)              # L2 B0
            hidden_block(0, 2, rhs_of_l1_cached, S1)    # L1 B2
            load_w(2)                                   # w3 into w1's slot
            hidden_block(1, 1, rhs_S1, S2)              # L2 B1
            hidden_block(1, 2, rhs_S1, S2)              # L2 B2 (frees S1 slot)
            hidden_block(2, 0, rhs_S2, S3)              # L3 B0
            hidden_block(2, 1, rhs_S2, S3)              # L3 B1
            lo0_tts = out_block(0, defer=True)          # Lo B0 (TTS woven)
            hidden_block(2, 2, rhs_S2, S3, filler=lo0_tts)  # L3 B2
            assert not lo0_tts
            out_block(1)                                # Lo B1
            out_block(2)                                # Lo B2

            # max over blocks, then over partitions; spikes are possible
            # only if the no-reset membrane crossed ~threshold somewhere
            nc.vector.tensor_reduce(
                mx1, mxb, mybir.AxisListType.X, ALU.max
            )
            nc.gpsimd.partition_all_reduce(
                mxa, mx1, 100, bass_isa.ReduceOp.max
            )
            # Engine-level conditional on the Vector engine (the whole
            # fallback is DVE-only): int32 bit-pattern compare -- positive
            # floats order like ints, negative floats read as negative ints
            # (condition false either way). 0.9375f leaves margin under the
            # spike threshold 1.0. (Fallback path HW-verified by forcing
            # the threshold to 0: identical output.)
            with tc.tile_critical():
                vreg = nc.vector.alloc_register("lomax_bits")
                nc.vector.reg_load(
                    vreg, mxa[0:1, 0:1].bitcast(mybir.dt.int32)
                )
                with nc.vector.If(nc.vector.snap(vreg) > 0x3F700000):
                    lo_scan_fallback()

            nc.sync.dma_start(out_d[:], ssum)

    nc.compile()
    return nc


_NC_CACHE = None


def _get_nc():
    global _NC_CACHE
    if _NC_CACHE is None:
        _NC_CACHE = _build_nc()
    return _NC_CACHE


def make_in_maps(x_seq, W1, b1, W2, b2, W3, b3, Wo, bo):
    f8 = ml_dtypes.float8_e4m3
    w1 = np.ascontiguousarray((W1 * WSCALE).astype(f8))
    w2 = np.ascontiguousarray((W2 * WSCALE).astype(f8))
    w3 = np.ascontiguousarray((W3 * WSCALE).astype(f8))
    wo_pad = np.zeros((H, 128), np.float32)
    wo_pad[:, :C] = Wo * WSCALE
    wo = np.ascontiguousarray(wo_pad.astype(f8))
    biases = np.concatenate(
        [b.reshape(HC, 128).T for b in (b1, b2, b3)], axis=1
    ).astype(np.float32)                       # [128, 48]
    biases = np.ascontiguousarray(biases)
    bo_a = np.ascontiguousarray(bo.reshape(C, 1).astype(np.float32))
    in_maps = []
    for c in range(NCORES):
        xs = x_seq[:, c * BC:(c + 1) * BC, :]              # [T, BC, D]
        xT = xs.transpose(2, 0, 1).reshape(KC, 128, R)     # [D,(t,b)] chunked
        in_maps.append({
            "xT": np.ascontiguousarray(xT.astype(f8)),
            "w1": w1, "w2": w2, "w3": w3, "wo": wo,
            "biases": biases, "biaso": bo_a,
        })
    return in_maps


def kernel(x_seq, W1, b1, W2, b2, W3, b3, Wo, bo):
    nc = _get_nc()
    in_maps = make_in_maps(x_seq, W1, b1, W2, b2, W3, b3, Wo, bo)
    res = run_bass_kernel_spmd(nc, in_maps, core_ids=list(range(NCORES)))
    outs = [res.results[c]["out"] for c in range(NCORES)]   # each [C, BC]
    return np.concatenate([o.T for o in outs], axis=0).astype(np.float32)
